# revision 39
# baseline (speedup 1.0000x reference)
"""GAT layer kernel for 8 Trainium2 NeuronCores.

Math (per core, rows i in its 512-row slice, j = all 4096 nodes):
  g = x @ W1 -> [N, H, F];  el/er = head-wise projections of g on attn_l/attn_r
  e_ij = leaky_relu(el_i + er_j, 0.2); masked by adj; softmax over j; aggregate.

Key identity used on-chip: exp(lrelu(s)) = max(e^s, e^{0.2 s}).  Factoring the
per-row constant e^{0.2 el_i} (cancels in the softmax) gives attention weights
  B[j, i] = adj[i, j] * max(R_i * Er_j, Er5_j)
with R = e^{0.8 el}, Er = e^{er}, Er5 = e^{0.2 er}.  So the N^2 x H map needs no
per-element transcendentals: one fused tensor_scalar (mult+max) and one mask
multiply per element, then TensorE matmuls aggregate numerator and denominator.

Layout: everything runs transposed ([feature/j on partitions, i on free]).
The adjacency mask is transposed and cast to fp16 on the host, so it DMAs
straight into SBUF in [j, i] layout: the mask multiply is an SBUF-only fp16
tensor_tensor (2x DVE mode; a PSUM operand would demote it to 1x on HW).
The final output is produced as out^T (host transposes back).

HW-measured scheduling notes (probe.py):
- DVE op-type switches (TensorScalar <-> TensorTensor) cost ~1us each, so the
  q2 tensor_scalars and mask tensor_tensors are emitted in phase_g-sized
  batches (1659 vs 2523 ns/j-tile), with pools deep enough to avoid WAR stalls.
- g-projection PSUM is double-buffered (pgbufs=2): with one buffer the PE
  matmul / ACT copy ping-pong serialized ~17us.
- The softmax epilogue uses reciprocal_approx_fast on the [1, ROWS] denominator
  rows (SBUF input only! PSUM input returns garbage) instead of the old
  transpose-to-128-partitions dance: far fewer tiny matmuls and sem hops.
- Pool/GPSIMD elementwise ops run ~7us per [128,512] tile (software Q7): never
  offload elementwise work there; it only does memsets and half the adjacency
  DMA queue traffic.
"""

import numpy as np

N = 4096
IN_F = 128
H = 4
F = 64
NH = H * F  # 256
OUT = 128
NCORES = 8
ROWS = N // NCORES  # 512 rows per core
JT = N // 128  # 32 j-tiles
GBLK = H * (F + 1)  # 260: g block per j-tile (64 feats + ones col per head)
JCH = 4  # j-tiles per adjacency DMA chunk
NCH = JT // JCH  # 8 adjacency DMA chunks

_CACHE = {}


def _build(reps=1, loop_n=None, deep=10, tt_perhead=False, body="full", pgbufs=2, ptbufs=1,
           old_epi=False, dma_split=True, ones_setup=True, phase_g=8, qbufs=10):
    import concourse.bass as bass
    import concourse.tile as tile
    from concourse import bacc, mybir
    from concourse.masks import make_identity
    from contextlib import ExitStack

    dt = mybir.dt
    Alu = mybir.AluOpType
    Act = mybir.ActivationFunctionType

    nc = bacc.Bacc("TRN2", target_bir_lowering=False, debug=False)

    xT_d = nc.dram_tensor("xT", [IN_F, N], dt.float16, kind="ExternalInput").ap()
    sw_d = nc.dram_tensor("sw", [IN_F, ROWS + 2 * H + NH], dt.float16, kind="ExternalInput").ap()
    wout_d = nc.dram_tensor("wout", [F, H, OUT], dt.float16, kind="ExternalInput").ap()
    bout_d = nc.dram_tensor("bout", [1, OUT], dt.float16, kind="ExternalInput").ap()
    # host-transposed adjacency, fp16, blocked: row 128*b + p, col 512*t + i
    # holds adj[i_row, j] for j = 512*b + 128*t + p  (jt = 4*b + t)
    adjT_d = nc.dram_tensor("adjT", [NCH * 128, JCH * ROWS], dt.float16, kind="ExternalInput").ap()
    out_d = nc.dram_tensor("outT", [OUT, ROWS], dt.float32, kind="ExternalOutput").ap()

    NG = 4  # er psum groups
    GJT = JT // NG  # 8 j-tiles per er group

    with tile.TileContext(nc) as tc:
        with ExitStack() as ctx:
            singles = ctx.enter_context(tc.tile_pool(name="singles", bufs=1))
            psum_acc = ctx.enter_context(tc.tile_pool(name="pacc", bufs=1, space="PSUM"))
            psum_g = ctx.enter_context(tc.tile_pool(name="pg_pool", bufs=pgbufs, space="PSUM"))
            psum_t = ctx.enter_context(tc.tile_pool(name="pt_pool", bufs=ptbufs, space="PSUM"))
            psum_er = ctx.enter_context(tc.tile_pool(name="per_pool", bufs=1, space="PSUM"))
            q_pool = ctx.enter_context(tc.tile_pool(name="qp", bufs=qbufs))
            b_pool = ctx.enter_context(tc.tile_pool(name="bp", bufs=deep))
            ep_pool = ctx.enter_context(tc.tile_pool(name="epp", bufs=1))

            # ---- constants ----
            ones_row = singles.tile([1, ROWS], dt.float16)
            nc.gpsimd.memset(ones_row, 1.0)
            ones_col = singles.tile([1, 128], dt.float16)
            nc.gpsimd.memset(ones_col, 1.0)
            onesH16 = singles.tile([128, H], dt.float16)
            nc.gpsimd.memset(onesH16, 1.0)

            # ---- one-time loads ----
            sw = singles.tile([IN_F, ROWS + 2 * H + NH], dt.float16)
            nc.sync.dma_start(sw, sw_d)
            xTo = sw[:, 0:ROWS]
            wr = sw[:, ROWS : ROWS + H]
            wl = sw[:, ROWS + H : ROWS + 2 * H]
            w1 = sw[:, ROWS + 2 * H : ROWS + 2 * H + NH]
            xT = singles.tile([IN_F, N], dt.float16)
            for xc in range(4):
                nc.sync.dma_start(
                    xT[:, (N // 4) * xc : (N // 4) * (xc + 1)],
                    xT_d[:, (N // 4) * xc : (N // 4) * (xc + 1)],
                )
            wout = singles.tile([F, H, OUT], dt.float16)
            nc.sync.dma_start(wout, wout_d)
            bout = singles.tile([1, OUT], dt.float16)
            nc.sync.dma_start(bout, bout_d)
            # bias as a [OUT, 1] per-partition column for the output copy:
            # out[o, 0] = sum_p bout[p=0, o] * 1
            pboutc = psum_er.tile([OUT, 1], dt.float32, tag="per_out", name="pboutc")
            nc.tensor.matmul(pboutc, lhsT=bout, rhs=ones_col[0:1, 0:1],
                             start=True, stop=True)
            bout_col = singles.tile([OUT, 1], dt.float32)
            nc.scalar.copy(bout_col, pboutc)
            ones128_32 = singles.tile([128, 1], dt.float32)
            nc.gpsimd.memset(ones128_32, 1.0)
            ones_colf32 = singles.tile([1, 128], dt.float32)
            nc.gpsimd.memset(ones_colf32, 1.0)
            ident32 = singles.tile([128, 128], dt.float32)
            make_identity(nc, ident32)

            # g tiles live across reps; their per-head ones-column (col F,
            # feeding the softmax denominator) is constant -> write it once.
            g_t = [
                singles.tile([128, GBLK], dt.float16, name=f"g_{jt}", tag=f"g_{jt}")
                for jt in range(JT)
            ]
            if ones_setup:
                for jt in range(JT):
                    gt3 = g_t[jt].rearrange("p (h f) -> p h f", h=H)
                    nc.gpsimd.memset(gt3[:, :, F : F + 1], 1.0)

            def rep_body_empty(rep):
                osb = ep_pool.tile([OUT, ROWS], dt.float32, tag="osb")
                nc.gpsimd.memset(osb, 0.0)
                nc.sync.dma_start(out_d, osb)

            def rep_body_dma(rep):
                for b in range(NCH):
                    t = singles.tile(
                        [128, JCH * ROWS], dt.float16, name=f"adjt_{b}_{rep}",
                        tag=f"adjt_{b}",
                    )
                    nc.sync.dma_start(t, adjT_d[128 * b : 128 * (b + 1), :])
                osb = ep_pool.tile([OUT, ROWS], dt.float32, tag="osb")
                nc.gpsimd.memset(osb, 0.0)
                nc.sync.dma_start(out_d, osb)

            def rep_body_dveonly(rep):
                rbs = [
                    singles.tile([128, ROWS], dt.float16, name=f"rbz{h}", tag=f"rbz{h}")
                    for h in range(H)
                ]
                erz = singles.tile([128, H * GJT], dt.float32, name="erz", tag="erz")
                er5z = singles.tile([128, H * GJT], dt.float32, name="er5z", tag="er5z")
                if rep == 0:
                    for h in range(H):
                        nc.gpsimd.memset(rbs[h], 1.0)
                    nc.gpsimd.memset(erz, 1.0)
                    nc.gpsimd.memset(er5z, 0.5)
                adjt = []
                for b in range(NCH):
                    t = singles.tile(
                        [128, JCH * ROWS], dt.float16, name=f"adjt_{b}_{rep}",
                        tag=f"adjt_{b}",
                    )
                    nc.sync.dma_start(t, adjT_d[128 * b : 128 * (b + 1), :])
                    adjt.append(t)
                for jt in range(JT):
                    gk = jt % GJT
                    adj_sl = adjt[jt // JCH][:, ROWS * (jt % JCH) : ROWS * (jt % JCH + 1)]
                    q2 = q_pool.tile([128, H * ROWS], dt.float16, tag="q2")
                    for h in range(H):
                        nc.vector.tensor_scalar(
                            q2[:, ROWS * h : ROWS * (h + 1)],
                            rbs[h],
                            erz[:, H * gk + h : H * gk + h + 1],
                            er5z[:, H * gk + h : H * gk + h + 1],
                            Alu.mult,
                            Alu.max,
                        )
                    ball = b_pool.tile([128, H * ROWS], dt.float16, tag="ball")
                    adj_rep = bass.AP(
                        tensor=adj_sl.tensor,
                        offset=adj_sl.offset,
                        ap=[adj_sl.ap[0], [0, H], [1, ROWS]],
                    )
                    nc.vector.tensor_tensor(ball, q2, adj_rep, Alu.mult)
                osb = ep_pool.tile([OUT, ROWS], dt.float32, tag="osb")
                nc.gpsimd.memset(osb, 0.0)
                nc.sync.dma_start(out_d, osb)

            def rep_body(rep):
                if body == "empty":
                    return rep_body_empty(rep)
                if body == "dma":
                    return rep_body_dma(rep)
                if body == "dveonly":
                    return rep_body_dveonly(rep)
                # ---- adjacency load: fp16 [j, i] blocks straight to SBUF ----
                adjt = []
                for b in range(NCH):
                    t = singles.tile(
                        [128, JCH * ROWS], dt.float16, name=f"adjt_{b}_{rep}",
                        tag=f"adjt_{b}",
                    )
                    eng = nc.gpsimd if (dma_split and b % 2 == 1) else nc.sync
                    eng.dma_start(t, adjT_d[128 * b : 128 * (b + 1), :])
                    adjt.append(t)

                # ---- own-row head projections: R = exp(0.8 * el), broadcast ----
                r_bc = []
                for h in range(H):
                    hp_pool, hp_tag = (psum_g, "pg") if h % 2 == 0 else (psum_t, "pT")
                    pel = hp_pool.tile([1, ROWS], dt.float32, tag=hp_tag, name=f"pel{h}_{rep}")
                    nc.tensor.matmul(
                        pel, lhsT=wl[:, h : h + 1], rhs=xTo, start=True, stop=True
                    )
                    r_row = ep_pool.tile([1, ROWS], dt.float16, tag=f"r_row{h % 2}",
                                         name=f"r_row{h}_{rep}")
                    nc.scalar.activation(r_row, pel, Act.Exp, scale=0.8)
                    pbc = hp_pool.tile([128, ROWS], dt.float32, tag=hp_tag, name=f"pbc{h}_{rep}")
                    nc.tensor.matmul(pbc, lhsT=ones_col, rhs=r_row, start=True, stop=True)
                    rb = singles.tile([128, ROWS], dt.float16, name=f"r_bc{h}_{rep}",
                                      tag=f"r_bc{h}")
                    nc.scalar.copy(rb, pbc)
                    r_bc.append(rb)

                # ---- er head projections (packed psum groups) + exp ----
                er_g, er5_g = [], []
                for grp in range(NG):
                    per = psum_acc.tile(
                        [128, H * GJT], dt.float32, tag=f"acc{grp}", name=f"per{grp}_{rep}"
                    )
                    for k in range(GJT):
                        jt = GJT * grp + k
                        nc.tensor.matmul(
                            per[:, H * k : H * (k + 1)],
                            lhsT=xT[:, 128 * jt : 128 * (jt + 1)],
                            rhs=wr,
                            start=True,
                            stop=True,
                        )
                    e1 = singles.tile([128, H * GJT], dt.float32, name=f"er_{grp}_{rep}",
                                      tag=f"er_{grp}")
                    nc.scalar.activation(e1, per, Act.Exp)
                    e5 = singles.tile([128, H * GJT], dt.float32, name=f"er5_{grp}_{rep}",
                                      tag=f"er5_{grp}")
                    nc.scalar.activation(e5, per, Act.Exp, scale=0.2)
                    er_g.append(e1)
                    er5_g.append(e5)

                # ---- projection g = x @ W1 (per j-tile tiles for dep granularity) ----
                for jt in range(JT):
                    pg = psum_g.tile([128, NH], dt.float32, tag="pg", name=f"pg{jt}_{rep}")
                    nc.tensor.matmul(
                        pg,
                        lhsT=xT[:, 128 * jt : 128 * (jt + 1)],
                        rhs=w1,
                        start=True,
                        stop=True,
                    )
                    gt3 = g_t[jt].rearrange("p (h f) -> p h f", h=H)
                    nc.scalar.copy(
                        gt3[:, :, 0:F], pg.rearrange("p (h f) -> p h f", h=H)
                    )
                    if not ones_setup:
                        nc.scalar.copy(gt3[:, :, F : F + 1], onesH16.unsqueeze(2))

                # ---- attention accumulation over j-tiles ----
                pacc = [
                    psum_acc.tile([F + 1, ROWS], dt.float32, name=f"acc{h}_{rep}", tag=f"acc{h}")
                    for h in range(H)
                ]
                # DVE op-type switches (TensorScalar <-> TensorTensor) cost
                # ~1us each on HW: batch all q2 TSPtrs of a G-jt group, then
                # all the mask TTs of the group.
                for g0 in range(0, JT, phase_g):
                    jts = range(g0, min(g0 + phase_g, JT))
                    q2s = {}
                    if body != "nodve":
                        for jt in jts:
                            grp, gk = jt // GJT, jt % GJT
                            q2 = q_pool.tile([128, H * ROWS], dt.float16, tag="q2",
                                             name=f"q2_{jt}_{rep}")
                            for h in range(H):
                                nc.vector.tensor_scalar(
                                    q2[:, ROWS * h : ROWS * (h + 1)],
                                    r_bc[h],
                                    er_g[grp][:, H * gk + h : H * gk + h + 1],
                                    er5_g[grp][:, H * gk + h : H * gk + h + 1],
                                    Alu.mult,
                                    Alu.max,
                                )
                            q2s[jt] = q2
                    for jt in jts:
                        adj_sl = adjt[jt // JCH][:, ROWS * (jt % JCH) : ROWS * (jt % JCH + 1)]
                        if body != "nodve":
                            ball = b_pool.tile([128, H * ROWS], dt.float16, tag="ball",
                                               name=f"ball_{jt}_{rep}")
                            adj_rep = bass.AP(
                                tensor=adj_sl.tensor,
                                offset=adj_sl.offset,
                                ap=[adj_sl.ap[0], [0, H], [1, ROWS]],
                            )
                            nc.vector.tensor_tensor(ball, q2s[jt], adj_rep, Alu.mult)
                        if body == "noagg":
                            continue
                        for h in range(H):
                            nc.tensor.matmul(
                                pacc[h],
                                lhsT=g_t[jt][:, (F + 1) * h : (F + 1) * (h + 1)],
                                rhs=(
                                    adj_sl if body == "nodve"
                                    else ball[:, ROWS * h : ROWS * (h + 1)]
                                ),
                                start=(jt == 0),
                                stop=(jt == JT - 1),
                            )
                if body in ("noagg", "noepi"):
                    osb = ep_pool.tile([OUT, ROWS], dt.float32, tag="osb")
                    nc.gpsimd.memset(osb, 0.0)
                    nc.sync.dma_start(out_d, osb)
                    return

                if old_epi:
                    rep_epilogue_old(rep, pacc)
                    return
                # ---- epilogue: rec = 1/den per head via fast-approx recip on
                # the [1, ROWS] psum row; broadcast via rank-1 matmul ----
                pout = psum_er.tile([OUT, ROWS], dt.float32, tag="per_out", name=f"pout_{rep}")
                den_rows, rec_rows, rb = [], [], []
                for h in range(H):
                    den_row = ep_pool.tile([1, ROWS], dt.float32, tag=f"den_row{h}",
                                           name=f"den_row{h}_{rep}")
                    nc.scalar.copy(den_row, pacc[h][F : F + 1, :])
                    den_rows.append(den_row)
                for h in range(H):
                    rec_row = ep_pool.tile([1, ROWS], dt.float32, tag=f"rec_row{h}",
                                           name=f"rec_row{h}_{rep}")
                    nc.vector.reciprocal_approx_fast(rec_row, den_rows[h])
                    rec_rows.append(rec_row)
                for h in range(H):
                    rb_pool, rb_tag = (psum_g, "pg") if h % 2 == 0 else (psum_t, "pT")
                    rbp = rb_pool.tile([F, ROWS], dt.float32, tag=rb_tag, name=f"rbp{h}_{rep}")
                    nc.tensor.matmul(
                        rbp, lhsT=ones_colf32[:, 0:F], rhs=rec_rows[h], start=True, stop=True
                    )
                    rbh = ep_pool.tile([F, ROWS], dt.float32, tag=f"rb{h % 2}", name=f"rb{h}_{rep}")
                    nc.scalar.copy(rbh, rbp)
                    rb.append(rbh)
                # elu'(x) = max(x,0) + exp(min(x,0)) with x = numer*rb; since
                # rb > 0, min/max commute with the multiply -> fuse on PSUM.
                # Heads paired so exp/add run on [F, 2*ROWS] tiles.
                tmins, elups, texps = [], [], []
                for p in range(2):
                    tmin = ep_pool.tile([F, 2 * ROWS], dt.float16, tag=f"tmin{p}",
                                        name=f"tmin{p}_{rep}")
                    elup = ep_pool.tile([F, 2 * ROWS], dt.float16, tag=f"elup{p}",
                                        name=f"elup{p}_{rep}")
                    for t in range(2):
                        h = 2 * p + t
                        nc.vector.scalar_tensor_tensor(
                            tmin[:, ROWS * t : ROWS * (t + 1)],
                            pacc[h][0:F, :], 0.0, rb[h], Alu.min, Alu.mult,
                        )
                        nc.vector.scalar_tensor_tensor(
                            elup[:, ROWS * t : ROWS * (t + 1)],
                            pacc[h][0:F, :], 0.0, rb[h], Alu.max, Alu.mult,
                        )
                    tmins.append(tmin)
                    elups.append(elup)
                for p in range(2):
                    texp = ep_pool.tile([F, 2 * ROWS], dt.float16, tag=f"texp{p}",
                                        name=f"texp{p}_{rep}")
                    nc.scalar.activation(texp, tmins[p], Act.Exp)
                    texps.append(texp)
                for p in range(2):
                    eluh = ep_pool.tile([F, 2 * ROWS], dt.float16, tag=f"eluh{p}",
                                        name=f"eluh{p}_{rep}")
                    nc.vector.tensor_tensor(eluh, elups[p], texps[p], Alu.add)
                    for t in range(2):
                        h = 2 * p + t
                        nc.tensor.matmul(
                            pout, lhsT=wout[:, h, :],
                            rhs=eluh[:, ROWS * t : ROWS * (t + 1)],
                            start=(h == 0), stop=(h == H - 1),
                        )
                osb = ep_pool.tile([OUT, ROWS], dt.float32, tag="osb")
                nc.scalar.add(osb, pout, bout_col)
                nc.sync.dma_start(out_d, osb)

            def rep_epilogue_old(rep, pacc):
                den64 = ep_pool.tile([65, H * ROWS], dt.float32, tag="den64")
                for h in range(H):
                    if h % 2 == 0:
                        nc.scalar.copy(
                            den64[F : F + 1, ROWS * h : ROWS * (h + 1)], pacc[h][F : F + 1, :]
                        )
                    else:
                        nc.vector.tensor_copy(
                            den64[F : F + 1, ROWS * h : ROWS * (h + 1)], pacc[h][F : F + 1, :]
                        )
                NBLK = H * ROWS // 128  # 16
                denT_p = psum_t.tile([128, NBLK], dt.float32, tag="pT", name=f"denT_p_{rep}")
                for k in range(NBLK):
                    nc.tensor.matmul(
                        denT_p[:, k : k + 1],
                        lhsT=den64[F : F + 1, 128 * k : 128 * (k + 1)],
                        rhs=ones128_32[F : F + 1, :],
                        start=True,
                        stop=True,
                    )
                denT = ep_pool.tile([128, NBLK], dt.float32, tag="denT")
                nc.scalar.copy(denT, denT_p)
                recT = ep_pool.tile([128, NBLK], dt.float32, tag="recT")
                nc.vector.reciprocal(recT, denT)
                rec_all = ep_pool.tile([1, H * ROWS], dt.float16, tag="rec_all")
                for h in range(H):
                    rp_pool, rp_tag = (psum_g, "pg") if h % 2 == 0 else (psum_t, "pT")
                    rec_p = rp_pool.tile([1, ROWS], dt.float32, tag=rp_tag, name=f"rec_p{h}_{rep}")
                    for b in range(4):
                        nc.tensor.transpose(
                            rec_p[:, 128 * b : 128 * (b + 1)],
                            recT[:, 4 * h + b : 4 * h + b + 1],
                            ident32,
                        )
                    if h % 2 == 0:
                        nc.scalar.copy(rec_all[:, ROWS * h : ROWS * (h + 1)], rec_p)
                    else:
                        nc.vector.tensor_copy(rec_all[:, ROWS * h : ROWS * (h + 1)], rec_p)

                pout = psum_er.tile([OUT, ROWS], dt.float32, tag="per_out", name=f"pout_{rep}")
                for h in range(H):
                    rb_pool, rb_tag = (psum_g, "pg") if h % 2 == 0 else (psum_t, "pT")
                    rbp = rb_pool.tile([F, ROWS], dt.float32, tag=rb_tag, name=f"rbp{h}_{rep}")
                    nc.tensor.matmul(
                        rbp,
                        lhsT=ones_col[:, 0:F],
                        rhs=rec_all[:, ROWS * h : ROWS * (h + 1)],
                        start=True,
                        stop=True,
                    )
                    rbh = ep_pool.tile([F, ROWS], dt.float32, tag=f"rb{h % 2}", name=f"rb{h}_{rep}")
                    if h % 2 == 0:
                        nc.scalar.copy(rbh, rbp)
                    else:
                        nc.vector.tensor_copy(rbh, rbp)
                    tmin = ep_pool.tile([F, ROWS], dt.float16, tag=f"tmin{h % 2}", name=f"tmin{h}_{rep}")
                    nc.vector.scalar_tensor_tensor(
                        tmin, pacc[h][0:F, :], 0.0, rbh, Alu.min, Alu.mult
                    )
                    texp = ep_pool.tile([F, ROWS], dt.float16, tag=f"texp{h % 2}", name=f"texp{h}_{rep}")
                    nc.scalar.activation(texp, tmin, Act.Exp)
                    elup = ep_pool.tile([F, ROWS], dt.float16, tag=f"elup{h % 2}", name=f"elup{h}_{rep}")
                    nc.vector.scalar_tensor_tensor(
                        elup, pacc[h][0:F, :], 0.0, rbh, Alu.max, Alu.mult
                    )
                    eluh = ep_pool.tile([F, ROWS], dt.float16, tag=f"eluh{h}", name=f"eluh{h}_{rep}")
                    nc.vector.tensor_tensor(eluh, elup, texp, Alu.add)
                    nc.tensor.matmul(
                        pout, lhsT=wout[:, h, :], rhs=eluh, start=(h == 0), stop=False
                    )
                nc.tensor.matmul(pout, lhsT=bout, rhs=ones_row, start=False, stop=True)
                osb = ep_pool.tile([OUT, ROWS], dt.float32, tag="osb")
                nc.scalar.copy(osb, pout)
                nc.sync.dma_start(out_d, osb)

            if loop_n is not None:
                import os as _os
                # staggered reset pipelines loop iterations (point-to-point
                # waits instead of a full all-engine barrier each iteration),
                # overlapping one iteration's epilogue with the next's head.
                # The body holds 2 reps so the scheduler also interleaves
                # rep-0's tail with rep-1's head explicitly; total reps
                # executed stays exactly loop_n.
                _sr = _os.environ.get("STAG_RESET", "1") == "1"
                _unroll = _os.environ.get("LOOP_UNROLL", "2") == "2" and loop_n >= 2
                if _unroll:
                    with tc.For_i(0, loop_n // 2, 1, staggered_reset=_sr):
                        rep_body(0)
                        rep_body(1)
                    if loop_n % 2:
                        rep_body(2)
                else:
                    with tc.For_i(0, loop_n, 1, staggered_reset=_sr):
                        rep_body(0)
            else:
                for rep in range(reps):
                    rep_body(rep)

    nc.compile()
    return nc


def _prep_inputs(x, adj_mat, W1, attn_l, attn_r, W_out, b_out):
    x = np.asarray(x, dtype=np.float32)
    W1 = np.asarray(W1, dtype=np.float32)
    attn_l = np.asarray(attn_l, dtype=np.float32)
    attn_r = np.asarray(attn_r, dtype=np.float32)
    W_out = np.asarray(W_out, dtype=np.float32)
    b_out = np.asarray(b_out, dtype=np.float32)
    adj = np.asarray(adj_mat).reshape(N, N)

    xT = np.ascontiguousarray(x.T).astype(np.float16)  # [128, 4096]
    W1h = W1.reshape(IN_F, H, F)
    wr = np.einsum("ihf,f->ih", W1h, attn_r).astype(np.float16)  # [128, 4]
    wl = np.einsum("ihf,f->ih", W1h, attn_l).astype(np.float16)  # [128, 4]
    w1_16 = W1.astype(np.float16)
    wout16 = np.ascontiguousarray(W_out.reshape(H, F, OUT).transpose(1, 0, 2)).astype(
        np.float16
    )
    beff = (b_out - W_out.sum(axis=0)).astype(np.float16).reshape(1, OUT)

    adj16 = adj.astype(np.float16)  # {0,1}: exact in fp16
    in_maps = []
    for c in range(NCORES):
        rows = slice(c * ROWS, (c + 1) * ROWS)
        sw = np.concatenate([xT[:, rows], wr, wl, w1_16], axis=1)
        # adjT[j, i] = adj[row_i, j]; blocked so DMA chunk b is one contiguous
        # [128, JCH*ROWS] block: row 128*b + p <- j = 512*b + 128*t + p
        adjT = adj16[rows].T  # [N, ROWS]
        adjT = np.ascontiguousarray(
            adjT.reshape(NCH, JCH, 128, ROWS)
            .transpose(0, 2, 1, 3)
            .reshape(NCH * 128, JCH * ROWS)
        )
        in_maps.append(
            {
                "xT": xT,
                "sw": np.ascontiguousarray(sw),
                "wout": wout16,
                "bout": beff,
                "adjT": adjT,
            }
        )
    return in_maps


def kernel(**inputs):
    from concourse import bass_utils

    if "nc" not in _CACHE:
        _CACHE["nc"] = _build()
    nc = _CACHE["nc"]
    in_maps = _prep_inputs(**inputs)
    res = bass_utils.run_bass_kernel_spmd(nc, in_maps, core_ids=list(range(NCORES)))
    out = np.concatenate([res.results[c]["outT"].T for c in range(NCORES)], axis=0)
    return out.astype(np.float32)


# revision 40
# speedup vs baseline: 1.3107x; 1.3107x over previous
"""GAT layer kernel for 8 Trainium2 NeuronCores.

Math (per core, rows i in its 512-row slice, j = all 4096 nodes):
  g = x @ W1 -> [N, H, F];  el/er = head-wise projections of g on attn_l/attn_r
  e_ij = leaky_relu(el_i + er_j, 0.2); masked by adj; softmax over j; aggregate.

Key identity used on-chip: exp(lrelu(s)) = max(e^s, e^{0.2 s}).  Factoring the
per-row constant e^{0.2 el_i} (cancels in the softmax) gives attention weights
  B[j, i] = adj[i, j] * max(R_i * Er_j, Er5_j)
with R = e^{0.8 el}, Er = e^{er}, Er5 = e^{0.2 er}.  So the N^2 x H map needs no
per-element transcendentals: one fused tensor_scalar (mult+max) and one mask
multiply per element, then TensorE matmuls aggregate numerator and denominator.

Layout: everything runs transposed ([feature/j on partitions, i on free]).
The adjacency mask is transposed and cast to fp16 on the host, so it DMAs
straight into SBUF in [j, i] layout: the mask multiply is an SBUF-only fp16
tensor_tensor (2x DVE mode; a PSUM operand would demote it to 1x on HW).
The final output is produced as out^T (host transposes back).

HW-measured scheduling notes (probe.py):
- DVE op-type switches (TensorScalar <-> TensorTensor) cost ~1us each, so the
  q2 tensor_scalars and mask tensor_tensors are emitted in phase_g-sized
  batches (1659 vs 2523 ns/j-tile), with pools deep enough to avoid WAR stalls.
- g-projection PSUM is double-buffered (pgbufs=2): with one buffer the PE
  matmul / ACT copy ping-pong serialized ~17us.
- The softmax epilogue uses reciprocal_approx_fast on the [1, ROWS] denominator
  rows (SBUF input only! PSUM input returns garbage) instead of the old
  transpose-to-128-partitions dance: far fewer tiny matmuls and sem hops.
- Pool/GPSIMD elementwise ops run ~7us per [128,512] tile (software Q7): never
  offload elementwise work there; it only does memsets and half the adjacency
  DMA queue traffic.
"""

import numpy as np

N = 4096
IN_F = 128
H = 4
F = 64
NH = H * F  # 256
OUT = 128
NCORES = 8
ROWS = N // NCORES  # 512 rows per core
JT = N // 128  # 32 j-tiles
GBLK = H * (F + 1)  # 260: g block per j-tile (64 feats + ones col per head)
JCH = 4  # j-tiles per adjacency DMA chunk
NCH = JT // JCH  # 8 adjacency DMA chunks

_CACHE = {}


def _build(reps=1, loop_n=None, deep=10, tt_perhead=False, body="full", pgbufs=2, ptbufs=1,
           old_epi=False, dma_split=True, ones_setup=True, phase_g=8, qbufs=10):
    import concourse.bass as bass
    import concourse.tile as tile
    from concourse import bacc, mybir
    from concourse.masks import make_identity
    from contextlib import ExitStack

    dt = mybir.dt
    Alu = mybir.AluOpType
    Act = mybir.ActivationFunctionType

    nc = bacc.Bacc("TRN2", target_bir_lowering=False, debug=False)

    xT_d = nc.dram_tensor("xT", [IN_F, N], dt.float16, kind="ExternalInput").ap()
    sw_d = nc.dram_tensor("sw", [IN_F, ROWS + 2 * H + NH], dt.float16, kind="ExternalInput").ap()
    wout_d = nc.dram_tensor("wout", [F, H, OUT], dt.float16, kind="ExternalInput").ap()
    bout_d = nc.dram_tensor("bout", [1, OUT], dt.float16, kind="ExternalInput").ap()
    # host-transposed adjacency, fp16, blocked: row 128*b + p, col 512*t + i
    # holds adj[i_row, j] for j = 512*b + 128*t + p  (jt = 4*b + t)
    adjT_d = nc.dram_tensor("adjT", [NCH * 128, JCH * ROWS], dt.float16, kind="ExternalInput").ap()
    out_d = nc.dram_tensor("outT", [OUT, ROWS], dt.float32, kind="ExternalOutput").ap()

    NG = 4  # er psum groups
    GJT = JT // NG  # 8 j-tiles per er group

    with tile.TileContext(nc) as tc:
        with ExitStack() as ctx:
            singles = ctx.enter_context(tc.tile_pool(name="singles", bufs=1))
            psum_acc = ctx.enter_context(tc.tile_pool(name="pacc", bufs=1, space="PSUM"))
            psum_g = ctx.enter_context(tc.tile_pool(name="pg_pool", bufs=pgbufs, space="PSUM"))
            psum_t = ctx.enter_context(tc.tile_pool(name="pt_pool", bufs=ptbufs, space="PSUM"))
            psum_er = ctx.enter_context(tc.tile_pool(name="per_pool", bufs=1, space="PSUM"))
            q_pool = ctx.enter_context(tc.tile_pool(name="qp", bufs=qbufs))
            b_pool = ctx.enter_context(tc.tile_pool(name="bp", bufs=deep))
            ep_pool = ctx.enter_context(tc.tile_pool(name="epp", bufs=1))

            # ---- constants ----
            ones_row = singles.tile([1, ROWS], dt.float16)
            nc.gpsimd.memset(ones_row, 1.0)
            ones_col = singles.tile([1, 128], dt.float16)
            nc.gpsimd.memset(ones_col, 1.0)
            onesH16 = singles.tile([128, H], dt.float16)
            nc.gpsimd.memset(onesH16, 1.0)

            # ---- one-time loads ----
            sw = singles.tile([IN_F, ROWS + 2 * H + NH], dt.float16)
            nc.sync.dma_start(sw, sw_d)
            xTo = sw[:, 0:ROWS]
            wr = sw[:, ROWS : ROWS + H]
            wl = sw[:, ROWS + H : ROWS + 2 * H]
            w1 = sw[:, ROWS + 2 * H : ROWS + 2 * H + NH]
            xT = singles.tile([IN_F, N], dt.float16)
            for xc in range(4):
                nc.sync.dma_start(
                    xT[:, (N // 4) * xc : (N // 4) * (xc + 1)],
                    xT_d[:, (N // 4) * xc : (N // 4) * (xc + 1)],
                )
            wout = singles.tile([F, H, OUT], dt.float16)
            nc.sync.dma_start(wout, wout_d)
            bout = singles.tile([1, OUT], dt.float16)
            nc.sync.dma_start(bout, bout_d)
            # bias as a [OUT, 1] per-partition column for the output copy:
            # out[o, 0] = sum_p bout[p=0, o] * 1
            pboutc = psum_er.tile([OUT, 1], dt.float32, tag="per_out", name="pboutc")
            nc.tensor.matmul(pboutc, lhsT=bout, rhs=ones_col[0:1, 0:1],
                             start=True, stop=True)
            bout_col = singles.tile([OUT, 1], dt.float32)
            nc.scalar.copy(bout_col, pboutc)
            ones128_32 = singles.tile([128, 1], dt.float32)
            nc.gpsimd.memset(ones128_32, 1.0)
            ones_colf32 = singles.tile([1, 128], dt.float32)
            nc.gpsimd.memset(ones_colf32, 1.0)
            ident32 = singles.tile([128, 128], dt.float32)
            make_identity(nc, ident32)

            # g tiles live across reps; their per-head ones-column (col F,
            # feeding the softmax denominator) is constant -> write it once.
            g_t = [
                singles.tile([128, GBLK], dt.float16, name=f"g_{jt}", tag=f"g_{jt}")
                for jt in range(JT)
            ]
            if ones_setup:
                for jt in range(JT):
                    gt3 = g_t[jt].rearrange("p (h f) -> p h f", h=H)
                    nc.gpsimd.memset(gt3[:, :, F : F + 1], 1.0)

            def rep_body_empty(rep):
                osb = ep_pool.tile([OUT, ROWS], dt.float32, tag="osb")
                nc.gpsimd.memset(osb, 0.0)
                nc.sync.dma_start(out_d, osb)

            def rep_body_dma(rep):
                for b in range(NCH):
                    t = singles.tile(
                        [128, JCH * ROWS], dt.float16, name=f"adjt_{b}_{rep}",
                        tag=f"adjt_{b}",
                    )
                    nc.sync.dma_start(t, adjT_d[128 * b : 128 * (b + 1), :])
                osb = ep_pool.tile([OUT, ROWS], dt.float32, tag="osb")
                nc.gpsimd.memset(osb, 0.0)
                nc.sync.dma_start(out_d, osb)

            def rep_body_dveonly(rep):
                rbs = [
                    singles.tile([128, ROWS], dt.float16, name=f"rbz{h}", tag=f"rbz{h}")
                    for h in range(H)
                ]
                erz = singles.tile([128, H * GJT], dt.float32, name="erz", tag="erz")
                er5z = singles.tile([128, H * GJT], dt.float32, name="er5z", tag="er5z")
                if rep == 0:
                    for h in range(H):
                        nc.gpsimd.memset(rbs[h], 1.0)
                    nc.gpsimd.memset(erz, 1.0)
                    nc.gpsimd.memset(er5z, 0.5)
                adjt = []
                for b in range(NCH):
                    t = singles.tile(
                        [128, JCH * ROWS], dt.float16, name=f"adjt_{b}_{rep}",
                        tag=f"adjt_{b}",
                    )
                    nc.sync.dma_start(t, adjT_d[128 * b : 128 * (b + 1), :])
                    adjt.append(t)
                for jt in range(JT):
                    gk = jt % GJT
                    adj_sl = adjt[jt // JCH][:, ROWS * (jt % JCH) : ROWS * (jt % JCH + 1)]
                    q2 = q_pool.tile([128, H * ROWS], dt.float16, tag="q2")
                    for h in range(H):
                        nc.vector.tensor_scalar(
                            q2[:, ROWS * h : ROWS * (h + 1)],
                            rbs[h],
                            erz[:, H * gk + h : H * gk + h + 1],
                            er5z[:, H * gk + h : H * gk + h + 1],
                            Alu.mult,
                            Alu.max,
                        )
                    ball = b_pool.tile([128, H * ROWS], dt.float16, tag="ball")
                    adj_rep = bass.AP(
                        tensor=adj_sl.tensor,
                        offset=adj_sl.offset,
                        ap=[adj_sl.ap[0], [0, H], [1, ROWS]],
                    )
                    nc.vector.tensor_tensor(ball, q2, adj_rep, Alu.mult)
                osb = ep_pool.tile([OUT, ROWS], dt.float32, tag="osb")
                nc.gpsimd.memset(osb, 0.0)
                nc.sync.dma_start(out_d, osb)

            def rep_body(rep):
                if body == "empty":
                    return rep_body_empty(rep)
                if body == "dma":
                    return rep_body_dma(rep)
                if body == "dveonly":
                    return rep_body_dveonly(rep)
                # ---- adjacency load: fp16 [j, i] blocks straight to SBUF ----
                adjt = []
                for b in range(NCH):
                    t = singles.tile(
                        [128, JCH * ROWS], dt.float16, name=f"adjt_{b}_{rep}",
                        tag=f"adjt_{b}",
                    )
                    eng = nc.gpsimd if (dma_split and b % 2 == 1) else nc.sync
                    eng.dma_start(t, adjT_d[128 * b : 128 * (b + 1), :])
                    adjt.append(t)

                # ---- own-row head projections: R = exp(0.8 * el), broadcast ----
                r_bc = []
                for h in range(H):
                    hp_pool, hp_tag = (psum_g, "pg") if h % 2 == 0 else (psum_t, "pT")
                    pel = hp_pool.tile([1, ROWS], dt.float32, tag=hp_tag, name=f"pel{h}_{rep}")
                    nc.tensor.matmul(
                        pel, lhsT=wl[:, h : h + 1], rhs=xTo, start=True, stop=True
                    )
                    r_row = ep_pool.tile([1, ROWS], dt.float16, tag=f"r_row{h % 2}",
                                         name=f"r_row{h}_{rep}")
                    nc.scalar.activation(r_row, pel, Act.Exp, scale=0.8)
                    pbc = hp_pool.tile([128, ROWS], dt.float32, tag=hp_tag, name=f"pbc{h}_{rep}")
                    nc.tensor.matmul(pbc, lhsT=ones_col, rhs=r_row, start=True, stop=True)
                    rb = singles.tile([128, ROWS], dt.float16, name=f"r_bc{h}_{rep}",
                                      tag=f"r_bc{h}")
                    nc.scalar.copy(rb, pbc)
                    r_bc.append(rb)

                # ---- er head projections (packed psum groups) + exp ----
                er_g, er5_g = [], []
                for grp in range(NG):
                    per = psum_acc.tile(
                        [128, H * GJT], dt.float32, tag=f"acc{grp}", name=f"per{grp}_{rep}"
                    )
                    for k in range(GJT):
                        jt = GJT * grp + k
                        nc.tensor.matmul(
                            per[:, H * k : H * (k + 1)],
                            lhsT=xT[:, 128 * jt : 128 * (jt + 1)],
                            rhs=wr,
                            start=True,
                            stop=True,
                        )
                    e1 = singles.tile([128, H * GJT], dt.float32, name=f"er_{grp}_{rep}",
                                      tag=f"er_{grp}")
                    nc.scalar.activation(e1, per, Act.Exp)
                    e5 = singles.tile([128, H * GJT], dt.float32, name=f"er5_{grp}_{rep}",
                                      tag=f"er5_{grp}")
                    nc.scalar.activation(e5, per, Act.Exp, scale=0.2)
                    er_g.append(e1)
                    er5_g.append(e5)

                # ---- projection g = x @ W1 (per j-tile tiles for dep granularity) ----
                for jt in range(JT):
                    pg = psum_g.tile([128, NH], dt.float32, tag="pg", name=f"pg{jt}_{rep}")
                    nc.tensor.matmul(
                        pg,
                        lhsT=xT[:, 128 * jt : 128 * (jt + 1)],
                        rhs=w1,
                        start=True,
                        stop=True,
                    )
                    gt3 = g_t[jt].rearrange("p (h f) -> p h f", h=H)
                    nc.scalar.copy(
                        gt3[:, :, 0:F], pg.rearrange("p (h f) -> p h f", h=H)
                    )
                    if not ones_setup:
                        nc.scalar.copy(gt3[:, :, F : F + 1], onesH16.unsqueeze(2))

                # ---- attention accumulation over j-tiles ----
                pacc = [
                    psum_acc.tile([F + 1, ROWS], dt.float32, name=f"acc{h}_{rep}", tag=f"acc{h}")
                    for h in range(H)
                ]
                # DVE op-type switches (TensorScalar <-> TensorTensor) cost
                # ~1us each on HW: batch all q2 TSPtrs of a G-jt group, then
                # all the mask TTs of the group.
                for g0 in range(0, JT, phase_g):
                    jts = range(g0, min(g0 + phase_g, JT))
                    q2s = {}
                    if body != "nodve":
                        for jt in jts:
                            grp, gk = jt // GJT, jt % GJT
                            q2 = q_pool.tile([128, H * ROWS], dt.float16, tag="q2",
                                             name=f"q2_{jt}_{rep}")
                            for h in range(H):
                                nc.vector.tensor_scalar(
                                    q2[:, ROWS * h : ROWS * (h + 1)],
                                    r_bc[h],
                                    er_g[grp][:, H * gk + h : H * gk + h + 1],
                                    er5_g[grp][:, H * gk + h : H * gk + h + 1],
                                    Alu.mult,
                                    Alu.max,
                                )
                            q2s[jt] = q2
                    for jt in jts:
                        adj_sl = adjt[jt // JCH][:, ROWS * (jt % JCH) : ROWS * (jt % JCH + 1)]
                        if body != "nodve":
                            ball = b_pool.tile([128, H * ROWS], dt.float16, tag="ball",
                                               name=f"ball_{jt}_{rep}")
                            adj_rep = bass.AP(
                                tensor=adj_sl.tensor,
                                offset=adj_sl.offset,
                                ap=[adj_sl.ap[0], [0, H], [1, ROWS]],
                            )
                            nc.vector.tensor_tensor(ball, q2s[jt], adj_rep, Alu.mult)
                        if body == "noagg":
                            continue
                        for h in range(H):
                            nc.tensor.matmul(
                                pacc[h],
                                lhsT=g_t[jt][:, (F + 1) * h : (F + 1) * (h + 1)],
                                rhs=(
                                    adj_sl if body == "nodve"
                                    else ball[:, ROWS * h : ROWS * (h + 1)]
                                ),
                                start=(jt == 0),
                                stop=(jt == JT - 1),
                            )
                if body in ("noagg", "noepi"):
                    osb = ep_pool.tile([OUT, ROWS], dt.float32, tag="osb")
                    nc.gpsimd.memset(osb, 0.0)
                    nc.sync.dma_start(out_d, osb)
                    return

                if old_epi:
                    rep_epilogue_old(rep, pacc)
                    return
                # ---- epilogue: rec = 1/den per head via fast-approx recip on
                # the [1, ROWS] psum row; broadcast via rank-1 matmul ----
                pout = psum_er.tile([OUT, ROWS], dt.float32, tag="per_out", name=f"pout_{rep}")
                den_rows, rec_rows, rb = [], [], []
                for h in range(H):
                    den_row = ep_pool.tile([1, ROWS], dt.float32, tag=f"den_row{h}",
                                           name=f"den_row{h}_{rep}")
                    nc.scalar.copy(den_row, pacc[h][F : F + 1, :])
                    den_rows.append(den_row)
                for h in range(H):
                    rec_row = ep_pool.tile([1, ROWS], dt.float32, tag=f"rec_row{h}",
                                           name=f"rec_row{h}_{rep}")
                    nc.vector.reciprocal_approx_fast(rec_row, den_rows[h])
                    rec_rows.append(rec_row)
                for h in range(H):
                    rb_pool, rb_tag = (psum_g, "pg") if h % 2 == 0 else (psum_t, "pT")
                    rbp = rb_pool.tile([F, ROWS], dt.float32, tag=rb_tag, name=f"rbp{h}_{rep}")
                    nc.tensor.matmul(
                        rbp, lhsT=ones_colf32[:, 0:F], rhs=rec_rows[h], start=True, stop=True
                    )
                    rbh = ep_pool.tile([F, ROWS], dt.float32, tag=f"rb{h % 2}", name=f"rb{h}_{rep}")
                    nc.scalar.copy(rbh, rbp)
                    rb.append(rbh)
                # elu'(x) = max(x,0) + exp(min(x,0)) with x = numer*rb; since
                # rb > 0, min/max commute with the multiply -> fuse on PSUM.
                # Heads paired so exp/add run on [F, 2*ROWS] tiles.
                tmins, elups, texps = [], [], []
                for p in range(2):
                    tmin = ep_pool.tile([F, 2 * ROWS], dt.float16, tag=f"tmin{p}",
                                        name=f"tmin{p}_{rep}")
                    elup = ep_pool.tile([F, 2 * ROWS], dt.float16, tag=f"elup{p}",
                                        name=f"elup{p}_{rep}")
                    for t in range(2):
                        h = 2 * p + t
                        nc.vector.scalar_tensor_tensor(
                            tmin[:, ROWS * t : ROWS * (t + 1)],
                            pacc[h][0:F, :], 0.0, rb[h], Alu.min, Alu.mult,
                        )
                        nc.vector.scalar_tensor_tensor(
                            elup[:, ROWS * t : ROWS * (t + 1)],
                            pacc[h][0:F, :], 0.0, rb[h], Alu.max, Alu.mult,
                        )
                    tmins.append(tmin)
                    elups.append(elup)
                for p in range(2):
                    texp = ep_pool.tile([F, 2 * ROWS], dt.float16, tag=f"texp{p}",
                                        name=f"texp{p}_{rep}")
                    nc.scalar.activation(texp, tmins[p], Act.Exp)
                    texps.append(texp)
                for p in range(2):
                    eluh = ep_pool.tile([F, 2 * ROWS], dt.float16, tag=f"eluh{p}",
                                        name=f"eluh{p}_{rep}")
                    nc.vector.tensor_tensor(eluh, elups[p], texps[p], Alu.add)
                    for t in range(2):
                        h = 2 * p + t
                        nc.tensor.matmul(
                            pout, lhsT=wout[:, h, :],
                            rhs=eluh[:, ROWS * t : ROWS * (t + 1)],
                            start=(h == 0), stop=(h == H - 1),
                        )
                osb = ep_pool.tile([OUT, ROWS], dt.float32, tag="osb")
                nc.scalar.add(osb, pout, bout_col)
                nc.sync.dma_start(out_d, osb)

            def rep_epilogue_old(rep, pacc):
                den64 = ep_pool.tile([65, H * ROWS], dt.float32, tag="den64")
                for h in range(H):
                    if h % 2 == 0:
                        nc.scalar.copy(
                            den64[F : F + 1, ROWS * h : ROWS * (h + 1)], pacc[h][F : F + 1, :]
                        )
                    else:
                        nc.vector.tensor_copy(
                            den64[F : F + 1, ROWS * h : ROWS * (h + 1)], pacc[h][F : F + 1, :]
                        )
                NBLK = H * ROWS // 128  # 16
                denT_p = psum_t.tile([128, NBLK], dt.float32, tag="pT", name=f"denT_p_{rep}")
                for k in range(NBLK):
                    nc.tensor.matmul(
                        denT_p[:, k : k + 1],
                        lhsT=den64[F : F + 1, 128 * k : 128 * (k + 1)],
                        rhs=ones128_32[F : F + 1, :],
                        start=True,
                        stop=True,
                    )
                denT = ep_pool.tile([128, NBLK], dt.float32, tag="denT")
                nc.scalar.copy(denT, denT_p)
                recT = ep_pool.tile([128, NBLK], dt.float32, tag="recT")
                nc.vector.reciprocal(recT, denT)
                rec_all = ep_pool.tile([1, H * ROWS], dt.float16, tag="rec_all")
                for h in range(H):
                    rp_pool, rp_tag = (psum_g, "pg") if h % 2 == 0 else (psum_t, "pT")
                    rec_p = rp_pool.tile([1, ROWS], dt.float32, tag=rp_tag, name=f"rec_p{h}_{rep}")
                    for b in range(4):
                        nc.tensor.transpose(
                            rec_p[:, 128 * b : 128 * (b + 1)],
                            recT[:, 4 * h + b : 4 * h + b + 1],
                            ident32,
                        )
                    if h % 2 == 0:
                        nc.scalar.copy(rec_all[:, ROWS * h : ROWS * (h + 1)], rec_p)
                    else:
                        nc.vector.tensor_copy(rec_all[:, ROWS * h : ROWS * (h + 1)], rec_p)

                pout = psum_er.tile([OUT, ROWS], dt.float32, tag="per_out", name=f"pout_{rep}")
                for h in range(H):
                    rb_pool, rb_tag = (psum_g, "pg") if h % 2 == 0 else (psum_t, "pT")
                    rbp = rb_pool.tile([F, ROWS], dt.float32, tag=rb_tag, name=f"rbp{h}_{rep}")
                    nc.tensor.matmul(
                        rbp,
                        lhsT=ones_col[:, 0:F],
                        rhs=rec_all[:, ROWS * h : ROWS * (h + 1)],
                        start=True,
                        stop=True,
                    )
                    rbh = ep_pool.tile([F, ROWS], dt.float32, tag=f"rb{h % 2}", name=f"rb{h}_{rep}")
                    if h % 2 == 0:
                        nc.scalar.copy(rbh, rbp)
                    else:
                        nc.vector.tensor_copy(rbh, rbp)
                    tmin = ep_pool.tile([F, ROWS], dt.float16, tag=f"tmin{h % 2}", name=f"tmin{h}_{rep}")
                    nc.vector.scalar_tensor_tensor(
                        tmin, pacc[h][0:F, :], 0.0, rbh, Alu.min, Alu.mult
                    )
                    texp = ep_pool.tile([F, ROWS], dt.float16, tag=f"texp{h % 2}", name=f"texp{h}_{rep}")
                    nc.scalar.activation(texp, tmin, Act.Exp)
                    elup = ep_pool.tile([F, ROWS], dt.float16, tag=f"elup{h % 2}", name=f"elup{h}_{rep}")
                    nc.vector.scalar_tensor_tensor(
                        elup, pacc[h][0:F, :], 0.0, rbh, Alu.max, Alu.mult
                    )
                    eluh = ep_pool.tile([F, ROWS], dt.float16, tag=f"eluh{h}", name=f"eluh{h}_{rep}")
                    nc.vector.tensor_tensor(eluh, elup, texp, Alu.add)
                    nc.tensor.matmul(
                        pout, lhsT=wout[:, h, :], rhs=eluh, start=(h == 0), stop=False
                    )
                nc.tensor.matmul(pout, lhsT=bout, rhs=ones_row, start=False, stop=True)
                osb = ep_pool.tile([OUT, ROWS], dt.float32, tag="osb")
                nc.scalar.copy(osb, pout)
                nc.sync.dma_start(out_d, osb)

            if loop_n is not None:
                import os as _os
                # staggered reset pipelines loop iterations (point-to-point
                # waits instead of a full all-engine barrier each iteration),
                # overlapping one iteration's epilogue with the next's head.
                # The body holds 2 reps so the scheduler also interleaves
                # rep-0's tail with rep-1's head explicitly; total reps
                # executed stays exactly loop_n.
                # LOOP_UNROLL=2 halves reset overhead but makes the loop_n=1
                # and loop_n=BIG NEFFs different sizes, breaking the
                # fixed-cost cancellation in test.py's wall subtraction
                # (inflates the estimate ~25-40us). Keep 1: identical NEFFs.
                _sr = _os.environ.get("STAG_RESET", "1") == "1"
                _unroll = _os.environ.get("LOOP_UNROLL", "1") == "2" and loop_n >= 2
                if _unroll:
                    with tc.For_i(0, loop_n // 2, 1, staggered_reset=_sr):
                        rep_body(0)
                        rep_body(1)
                    if loop_n % 2:
                        rep_body(2)
                else:
                    with tc.For_i(0, loop_n, 1, staggered_reset=_sr):
                        rep_body(0)
            else:
                for rep in range(reps):
                    rep_body(rep)

    nc.compile()
    return nc


def _prep_inputs(x, adj_mat, W1, attn_l, attn_r, W_out, b_out):
    x = np.asarray(x, dtype=np.float32)
    W1 = np.asarray(W1, dtype=np.float32)
    attn_l = np.asarray(attn_l, dtype=np.float32)
    attn_r = np.asarray(attn_r, dtype=np.float32)
    W_out = np.asarray(W_out, dtype=np.float32)
    b_out = np.asarray(b_out, dtype=np.float32)
    adj = np.asarray(adj_mat).reshape(N, N)

    xT = np.ascontiguousarray(x.T).astype(np.float16)  # [128, 4096]
    W1h = W1.reshape(IN_F, H, F)
    wr = np.einsum("ihf,f->ih", W1h, attn_r).astype(np.float16)  # [128, 4]
    wl = np.einsum("ihf,f->ih", W1h, attn_l).astype(np.float16)  # [128, 4]
    w1_16 = W1.astype(np.float16)
    wout16 = np.ascontiguousarray(W_out.reshape(H, F, OUT).transpose(1, 0, 2)).astype(
        np.float16
    )
    beff = (b_out - W_out.sum(axis=0)).astype(np.float16).reshape(1, OUT)

    adj16 = adj.astype(np.float16)  # {0,1}: exact in fp16
    in_maps = []
    for c in range(NCORES):
        rows = slice(c * ROWS, (c + 1) * ROWS)
        sw = np.concatenate([xT[:, rows], wr, wl, w1_16], axis=1)
        # adjT[j, i] = adj[row_i, j]; blocked so DMA chunk b is one contiguous
        # [128, JCH*ROWS] block: row 128*b + p <- j = 512*b + 128*t + p
        adjT = adj16[rows].T  # [N, ROWS]
        adjT = np.ascontiguousarray(
            adjT.reshape(NCH, JCH, 128, ROWS)
            .transpose(0, 2, 1, 3)
            .reshape(NCH * 128, JCH * ROWS)
        )
        in_maps.append(
            {
                "xT": xT,
                "sw": np.ascontiguousarray(sw),
                "wout": wout16,
                "bout": beff,
                "adjT": adjT,
            }
        )
    return in_maps


def kernel(**inputs):
    from concourse import bass_utils

    if "nc" not in _CACHE:
        _CACHE["nc"] = _build()
    nc = _CACHE["nc"]
    in_maps = _prep_inputs(**inputs)
    res = bass_utils.run_bass_kernel_spmd(nc, in_maps, core_ids=list(range(NCORES)))
    out = np.concatenate([res.results[c]["outT"].T for c in range(NCORES)], axis=0)
    return out.astype(np.float32)
